# revision 47
# baseline (speedup 1.0000x reference)
"""Trainium2 Bass kernel for gated-attention MIL pooling (batched).

Reference computation (per bag b of B=32, N=4096 instances, L=512 feats, D=256):
    a = tanh(x @ Wa + ba); g = sigmoid(x @ Wb + bb)
    s = (a*g) @ Wc + bc            # [N]  (bc dropped: softmax shift-invariant)
    A = softmax(s)                 # over N
    pooled = A @ x                 # [L]
    returns (A [B,N], x_batch passthrough, pooled [B,L])

Sharding: data-parallel over B across 8 cores (4 bags/core). Host passes x both
natural and pre-transposed (bf16) so the contraction dim lands on SBUF
partitions without any on-device transpose. sigmoid(z) = 0.5 + 0.5*tanh(z/2)
keeps tanh+exp on one ACT table set; the 0.5 factors fold into Wc and bb.
Wc is folded into the gating on DVE (hsum = sum_dc h_dc*wc_dc), so scores are
one partition-sum matmul (vs ones) per n-tile, landing n-on-partitions for the
softmax and the pooled matmuls. Softmax normalization (1/Z) happens on host
from the raw exp outputs. Bag tails (score/exp/pooled) are software-pipelined
into the next bag's main matmuls so the PE never waits on the DVE/ACT chain.
"""

import numpy as np
import ml_dtypes

import concourse.bacc as bacc
import concourse.tile as tile
from concourse import mybir
from concourse import bass_utils

B, N, L, D = 32, 4096, 512, 256
NCORES = 8
BAGS = B // NCORES          # 4 bags per core
NT = N // 128               # 32 n-tiles per bag
LC = L // 128               # 4 l-chunks
DC = D // 128               # 2 d-chunks
NQ = 4                      # n quarters (one psum phase + one xT DMA each)
QW = N // NQ                # 1024 columns per quarter

BF16 = mybir.dt.bfloat16
F32 = mybir.dt.float32

_cache = {}


def _build(repeat=1, variant="full"):
    do_mm = variant in ("full", "mm", "nodve")
    do_act = variant in ("full", "nodve")
    do_rest = variant == "full"
    nc = bacc.Bacc("TRN2", target_bir_lowering=False, debug=False,
                   num_devices=NCORES)
    xT = nc.dram_tensor("xT", [BAGS, L, N], BF16, kind="ExternalInput").ap()
    xn = nc.dram_tensor("xn", [BAGS, N, L], BF16, kind="ExternalInput").ap()
    w = nc.dram_tensor("w", [128, 2, LC, D], BF16, kind="ExternalInput").ap()
    bias = nc.dram_tensor("bias", [128, 2, DC], F32, kind="ExternalInput").ap()
    wc = nc.dram_tensor("wc", [128, DC], F32, kind="ExternalInput").ap()
    e_out = nc.dram_tensor("e_out", [BAGS, 128, NT], F32,
                           kind="ExternalOutput").ap()
    z_out = nc.dram_tensor("z_out", [BAGS, 128, 1], F32,
                           kind="ExternalOutput").ap()
    pooled_out = nc.dram_tensor("pooled_out", [1, BAGS * L], F32,
                                kind="ExternalOutput").ap()

    bags = [b for _ in range(repeat) for b in range(BAGS)]

    with tile.TileContext(nc) as tc:
        with (
            tc.tile_pool(name="singles", bufs=1) as singles,
            tc.tile_pool(name="xt", bufs=2) as xt_pool,
            tc.tile_pool(name="xn", bufs=2) as xn_pool,
            tc.tile_pool(name="acts", bufs=1) as acts,
            tc.tile_pool(name="hsums", bufs=2) as hsums,
            tc.tile_pool(name="small", bufs=2) as small,
            tc.tile_pool(name="pmain", bufs=2, space="PSUM") as pmain,
            tc.tile_pool(name="pscore", bufs=2, space="PSUM") as pscore,
            tc.tile_pool(name="ppool", bufs=2, space="PSUM") as ppool,
        ):
            # load order tuned for time-to-first-matmul: weights (small)
            # first, then the first x quarter
            w_sb = singles.tile([128, 2, LC, D], BF16)
            nc.sync.dma_start(out=w_sb, in_=w)
            xt_pre = xt_pool.tile([128, LC, QW], BF16, tag="xt0")
            nc.sync.dma_start(
                out=xt_pre,
                in_=xT[bags[0], :, 0:QW].rearrange("(c p) n -> p c n", p=128),
            )
            bias_sb = singles.tile([128, 2, DC], F32)
            nc.sync.dma_start(out=bias_sb, in_=bias)
            wc_sb = singles.tile([128, DC], F32)
            nc.sync.dma_start(out=wc_sb, in_=wc)
            ones_bf = singles.tile([128, 1], BF16)
            nc.vector.memset(ones_bf, 1.0)
            pooled_sb = singles.tile([1, BAGS * L], F32)
            if not do_rest:
                nc.vector.memset(pooled_sb, 0.0)

            # pending tail state: (bag, hsum_tile, xn_tile, e_bf_tile)
            pend = {}

            def emit_score(p):
                """Scores for pending bag: s[p, ti] = sum_part hsum[:, ti*128+p],
                then e = exp(s) (+ row partials), e cast to bf16. If "hsum2"
                is present (last bag), accumulate two partial tensors so the
                first half of the matmuls only depends on the dc=0 gating."""
                s_ps = pscore.tile([128, NT], F32)
                parts = [p["hsum"]] + ([p["hsum2"]] if "hsum2" in p else [])
                for k, hs in enumerate(parts):
                    for ti in range(NT):
                        nc.tensor.matmul(
                            s_ps[:, ti:ti + 1],
                            lhsT=hs[:, ti * 128:(ti + 1) * 128],
                            rhs=ones_bf,
                            start=(k == 0), stop=(k == len(parts) - 1),
                        )
                e_sb = small.tile([128, NT], F32, tag="e")
                part = small.tile([128, 1], F32, tag="part")
                nc.scalar.activation(
                    out=e_sb, in_=s_ps,
                    func=mybir.ActivationFunctionType.Exp,
                    accum_out=part,
                )
                e_bf = small.tile([128, NT], BF16, tag="ebf")
                nc.vector.tensor_copy(e_bf, e_sb)
                nc.sync.dma_start(out=e_out[p["bag"]], in_=e_sb)
                nc.sync.dma_start(out=z_out[p["bag"]], in_=part)
                p["ebf"] = e_bf

            def emit_pooled(p):
                # 4-way column-tiled accumulation: group j handles tiles
                # ti ≡ j (mod 4), concurrent in PE col-groups, partial sums on
                # psum partitions 0/32/64/96; combined with 3 DVE adds.
                p_ps = ppool.tile([128, L], F32, tag="p_ps")
                for ti in range(NT):
                    j = ti % 4
                    nc.tensor.matmul(
                        p_ps[32 * j:32 * j + 1, :],
                        lhsT=p["ebf"][:, ti:ti + 1],
                        rhs=p["xn"][:, ti],
                        start=(ti < 4), stop=(ti >= NT - 4),
                        tile_position=(0, 32 * j),
                    )
                acc = small.tile([1, L], F32, tag="pacc")
                nc.vector.tensor_copy(acc, p_ps[0:1, :])
                nc.vector.tensor_add(acc, acc, p_ps[32:33, :])
                nc.vector.tensor_add(acc, acc, p_ps[64:65, :])
                nc.vector.tensor_add(
                    pooled_sb[:, p["bag"] * L:(p["bag"] + 1) * L],
                    acc, p_ps[96:97, :])

            for it, bag in enumerate(bags):
                xt_sb = []
                for q in range(NQ):
                    if it == 0 and q == 0:
                        xt_sb.append(xt_pre)
                        continue
                    t_ = xt_pool.tile([128, LC, QW], BF16, tag=f"xt{q}")
                    nc.sync.dma_start(
                        out=t_,
                        in_=xT[bag, :, q * QW:(q + 1) * QW]
                        .rearrange("(c p) n -> p c n", p=128),
                    )
                    xt_sb.append(t_)
                # natural-layout x for the PENDING bag's pooled matmuls (which
                # run during this bag's mains) — deferring it keeps this bag's
                # xT quarters at the head of the DMA queue
                if pend or not do_rest:
                    xn_sb = xn_pool.tile([128, NT, L], BF16, tag="xn")
                    nc.sync.dma_start(
                        out=xn_sb,
                        in_=xn[pend["bag"] if pend else bag]
                        .rearrange("(t p) l -> p t l", p=128))
                    if pend:
                        pend["xn"] = xn_sb
                if do_rest and it == len(bags) - 1:
                    # the final bag's xn, loaded now so its pooled tail
                    # (emitted after the loop) doesn't wait on DMA
                    xn_last = xn_pool.tile([128, NT, L], BF16, tag="xn")
                    nc.sync.dma_start(
                        out=xn_last,
                        in_=xn[bag].rearrange("(t p) l -> p t l", p=128))

                a_sb = acts.tile([128, DC, N], BF16, tag="a")
                t_sb = acts.tile([128, DC, N], BF16, tag="t")
                hsum = None
                if do_rest:
                    hsum = hsums.tile([128, N], BF16, tag="hsum")
                # main matmuls: out[d, n] = sum_l W[l, d] * xT[l, n]
                for wi in range(2):
                    if not do_mm:
                        break
                    dst = a_sb if wi == 0 else t_sb
                    scale = 1.0 if wi == 0 else 0.5
                    for dc in range(DC):
                        for q in range(NQ):
                            ps = pmain.tile([128, 2, 512], F32)
                            for lc in range(LC):
                                for i in range(2):
                                    nc.tensor.matmul(
                                        ps[:, i],
                                        lhsT=w_sb[:, wi, lc,
                                                  dc * 128:(dc + 1) * 128],
                                        rhs=xt_sb[q][:, lc,
                                                     i * 512:(i + 1) * 512],
                                        start=(lc == 0), stop=(lc == LC - 1),
                                    )
                            if do_act:
                                nc.scalar.activation(
                                    out=dst[:, dc, q * QW:(q + 1) * QW],
                                    in_=ps[:].rearrange("p a b -> p (a b)"),
                                    func=mybir.ActivationFunctionType.Tanh,
                                    bias=bias_sb[:, wi, dc:dc + 1],
                                    scale=scale,
                                )
                        # gating as soon as this dc's tanh pair is complete:
                        # h = a*(1+t), hsum += h*wc_dc   (all bf16, in place;
                        # last bag keeps the dc parts separate so emit_score
                        # can start on the dc=0 half sooner)
                        if do_rest and wi == 1:
                            last = it == len(bags) - 1
                            nc.vector.tensor_mul(t_sb[:, dc], a_sb[:, dc],
                                                 t_sb[:, dc])
                            nc.vector.tensor_add(t_sb[:, dc], a_sb[:, dc],
                                                 t_sb[:, dc])
                            if dc == 0:
                                nc.vector.tensor_scalar_mul(
                                    hsum, t_sb[:, 0], wc_sb[:, 0:1])
                            else:
                                nc.vector.tensor_scalar_mul(
                                    t_sb[:, 1], t_sb[:, 1], wc_sb[:, 1:2])
                                nc.vector.tensor_add(hsum, hsum,
                                                     t_sb[:, 1])
                    # interleave the previous bag's tail into this bag's mains
                    if do_rest and pend:
                        if wi == 0:
                            emit_score(pend)
                        else:
                            emit_pooled(pend)

                if not do_rest:
                    continue

                pend = {"bag": bag, "hsum": hsum}

            if do_rest and pend:
                pend["xn"] = xn_last
                emit_score(pend)
                emit_pooled(pend)
            nc.sync.dma_start(out=pooled_out, in_=pooled_sb)
    nc.finalize()
    return nc


def _get_nc():
    if "nc" not in _cache:
        _cache["nc"] = _build()
    return _cache["nc"]


def kernel(x_batch, Wa, ba, Wb, bb, Wc, bc):
    x_batch = np.asarray(x_batch)
    Wa, ba = np.asarray(Wa, np.float32), np.asarray(ba, np.float32)
    Wb, bb = np.asarray(Wb, np.float32), np.asarray(bb, np.float32)
    Wc = np.asarray(Wc, np.float32)
    in_dtype = x_batch.dtype
    xf = np.asarray(x_batch, np.float32)

    bf16 = ml_dtypes.bfloat16
    # stationary weights: [p, wi, lc, d] = W_wi[lc*128+p, d]
    wstack = np.stack([Wa, Wb]).reshape(2, LC, 128, D).transpose(2, 0, 1, 3)
    w_host = np.ascontiguousarray(wstack).astype(bf16)
    # bias: [p, wi, dc]; the b-branch uses tanh(z/2) -> bias = bb/2
    bias_host = np.stack([ba, 0.5 * bb]).reshape(2, DC, 128).transpose(2, 0, 1)
    bias_host = np.ascontiguousarray(bias_host, np.float32)
    # wc: [p, dc] = 0.5 * Wc[dc*128+p]  (0.5 from the sigmoid-as-tanh identity)
    wc_host = np.ascontiguousarray(
        (0.5 * Wc[:, 0]).reshape(DC, 128).T, dtype=np.float32)

    xb = xf.astype(bf16)                      # [B, N, L]
    in_maps = []
    for c in range(NCORES):
        sh = xb[c * BAGS:(c + 1) * BAGS]
        in_maps.append({
            "xT": np.ascontiguousarray(sh.transpose(0, 2, 1)),
            "xn": np.ascontiguousarray(sh),
            "w": w_host,
            "bias": bias_host,
            "wc": wc_host,
        })

    nc = _get_nc()
    res = bass_utils.run_bass_kernel_spmd(nc, in_maps,
                                          core_ids=list(range(NCORES)))

    A = np.empty((B, N), np.float32)
    pooled = np.empty((B, L), np.float32)
    for c in range(NCORES):
        r = res.results[c]
        e = r["e_out"]                        # [BAGS, 128, NT]
        z = r["z_out"]                        # [BAGS, 128, 1]
        pr = r["pooled_out"].reshape(BAGS, L)
        for b in range(BAGS):
            bag = c * BAGS + b
            Z = z[b].sum(dtype=np.float64)
            A[bag] = (e[b].T.reshape(N) / Z).astype(np.float32)
            pooled[bag] = (pr[b] / Z).astype(np.float32)

    return (A.astype(in_dtype, copy=False), x_batch,
            pooled.astype(in_dtype, copy=False))


if __name__ == "__main__":
    rng = np.random.default_rng(0)
    inputs = {
        "x_batch": rng.standard_normal((B, N, L), dtype=np.float32),
        "Wa": (rng.uniform(-1, 1, (L, D)) / np.sqrt(L)).astype(np.float32),
        "ba": (rng.uniform(-1, 1, (D,)) / np.sqrt(L)).astype(np.float32),
        "Wb": (rng.uniform(-1, 1, (L, D)) / np.sqrt(L)).astype(np.float32),
        "bb": (rng.uniform(-1, 1, (D,)) / np.sqrt(L)).astype(np.float32),
        "Wc": (rng.uniform(-1, 1, (D, 1)) / np.sqrt(D)).astype(np.float32),
        "bc": (rng.uniform(-1, 1, (1,)) / np.sqrt(D)).astype(np.float32),
    }
    A, xb, pooled = kernel(**inputs)
    print("A", A.shape, A.dtype, "pooled", pooled.shape, pooled.dtype)


# revision 50
# speedup vs baseline: 1.0026x; 1.0026x over previous
"""Trainium2 Bass kernel for gated-attention MIL pooling (batched).

Reference computation (per bag b of B=32, N=4096 instances, L=512 feats, D=256):
    a = tanh(x @ Wa + ba); g = sigmoid(x @ Wb + bb)
    s = (a*g) @ Wc + bc            # [N]  (bc dropped: softmax shift-invariant)
    A = softmax(s)                 # over N
    pooled = A @ x                 # [L]
    returns (A [B,N], x_batch passthrough, pooled [B,L])

Sharding: data-parallel over B across 8 cores (4 bags/core). Host passes x both
natural and pre-transposed (bf16) so the contraction dim lands on SBUF
partitions without any on-device transpose. sigmoid(z) = 0.5 + 0.5*tanh(z/2)
keeps tanh+exp on one ACT table set; the 0.5 factors fold into Wc and bb.
Wc is folded into the gating on DVE (hsum = sum_dc h_dc*wc_dc), so scores are
one partition-sum matmul (vs ones) per n-tile, landing n-on-partitions for the
softmax and the pooled matmuls. Softmax normalization (1/Z) happens on host
from the raw exp outputs. Bag tails (score/exp/pooled) are software-pipelined
into the next bag's main matmuls so the PE never waits on the DVE/ACT chain.
"""

import numpy as np
import ml_dtypes

import concourse.bacc as bacc
import concourse.tile as tile
from concourse import mybir
from concourse import bass_utils

B, N, L, D = 32, 4096, 512, 256
NCORES = 8
BAGS = B // NCORES          # 4 bags per core
NT = N // 128               # 32 n-tiles per bag
LC = L // 128               # 4 l-chunks
DC = D // 128               # 2 d-chunks
NQ = 4                      # n quarters (one psum phase + one xT DMA each)
QW = N // NQ                # 1024 columns per quarter

BF16 = mybir.dt.bfloat16
F32 = mybir.dt.float32

_cache = {}


def _build(repeat=1, variant="full"):
    do_mm = variant in ("full", "mm", "nodve")
    do_act = variant in ("full", "nodve")
    do_rest = variant == "full"
    nc = bacc.Bacc("TRN2", target_bir_lowering=False, debug=False,
                   num_devices=NCORES)
    xT = nc.dram_tensor("xT", [BAGS, L, N], BF16, kind="ExternalInput").ap()
    xn = nc.dram_tensor("xn", [BAGS, N, L], BF16, kind="ExternalInput").ap()
    w = nc.dram_tensor("w", [128, 2, LC, D], BF16, kind="ExternalInput").ap()
    bias = nc.dram_tensor("bias", [128, 2, DC], F32, kind="ExternalInput").ap()
    wc = nc.dram_tensor("wc", [128, DC], F32, kind="ExternalInput").ap()
    e_out = nc.dram_tensor("e_out", [BAGS, 128, NT], F32,
                           kind="ExternalOutput").ap()
    z_out = nc.dram_tensor("z_out", [BAGS, 128, 1], F32,
                           kind="ExternalOutput").ap()
    pooled_out = nc.dram_tensor("pooled_out", [1, BAGS * L], F32,
                                kind="ExternalOutput").ap()

    bags = [b for _ in range(repeat) for b in range(BAGS)]

    with tile.TileContext(nc) as tc:
        with (
            tc.tile_pool(name="singles", bufs=1) as singles,
            tc.tile_pool(name="xt", bufs=2) as xt_pool,
            tc.tile_pool(name="xn", bufs=2) as xn_pool,
            tc.tile_pool(name="acts", bufs=1) as acts,
            tc.tile_pool(name="hsums", bufs=2) as hsums,
            tc.tile_pool(name="small", bufs=2) as small,
            tc.tile_pool(name="pmain", bufs=2, space="PSUM") as pmain,
            tc.tile_pool(name="pscore", bufs=2, space="PSUM") as pscore,
            tc.tile_pool(name="ppool", bufs=2, space="PSUM") as ppool,
        ):
            # load order tuned for time-to-first-matmul: weights (small)
            # first, then the first x quarter
            w_sb = singles.tile([128, 2, LC, D], BF16)
            nc.sync.dma_start(out=w_sb, in_=w)
            xt_pre = xt_pool.tile([128, LC, QW], BF16, tag="xt0")
            nc.sync.dma_start(
                out=xt_pre,
                in_=xT[bags[0], :, 0:QW].rearrange("(c p) n -> p c n", p=128),
            )
            bias_sb = singles.tile([128, 2, DC], F32)
            nc.sync.dma_start(out=bias_sb, in_=bias)
            wc_sb = singles.tile([128, DC], F32)
            nc.sync.dma_start(out=wc_sb, in_=wc)
            ones_bf = singles.tile([128, 1], BF16)
            nc.vector.memset(ones_bf, 1.0)
            pooled_sb = singles.tile([1, BAGS * L], F32)
            if not do_rest:
                nc.vector.memset(pooled_sb, 0.0)

            # pending tail state: (bag, hsum_tile, xn_tile, e_bf_tile)
            pend = {}

            def emit_score(p):
                """Scores for pending bag: s[p, ti] = sum_part hsum[:, ti*128+p],
                then e = exp(s) (+ row partials), e cast to bf16. If "hsum2"
                is present (last bag), accumulate two partial tensors so the
                first half of the matmuls only depends on the dc=0 gating."""
                s_ps = pscore.tile([128, NT], F32)
                parts = [p["hsum"]] + ([p["hsum2"]] if "hsum2" in p else [])
                for k, hs in enumerate(parts):
                    for ti in range(NT):
                        nc.tensor.matmul(
                            s_ps[:, ti:ti + 1],
                            lhsT=hs[:, ti * 128:(ti + 1) * 128],
                            rhs=ones_bf,
                            start=(k == 0), stop=(k == len(parts) - 1),
                        )
                e_sb = small.tile([128, NT], F32, tag="e")
                part = small.tile([128, 1], F32, tag="part")
                nc.scalar.activation(
                    out=e_sb, in_=s_ps,
                    func=mybir.ActivationFunctionType.Exp,
                    accum_out=part,
                )
                e_bf = small.tile([128, NT], BF16, tag="ebf")
                nc.vector.tensor_copy(e_bf, e_sb)
                nc.sync.dma_start(out=e_out[p["bag"]], in_=e_sb)
                nc.sync.dma_start(out=z_out[p["bag"]], in_=part)
                p["ebf"] = e_bf

            def emit_pooled(p):
                # 4-way column-tiled accumulation: group j handles tiles
                # ti ≡ j (mod 4), concurrent in PE col-groups, partial sums on
                # psum partitions 0/32/64/96; combined with 3 DVE adds.
                p_ps = ppool.tile([128, L], F32, tag="p_ps")
                for ti in range(NT):
                    j = ti % 4
                    nc.tensor.matmul(
                        p_ps[32 * j:32 * j + 1, :],
                        lhsT=p["ebf"][:, ti:ti + 1],
                        rhs=p["xn"][:, ti],
                        start=(ti < 4), stop=(ti >= NT - 4),
                        tile_position=(0, 32 * j),
                    )
                acc = small.tile([1, L], F32, tag="pacc")
                nc.vector.tensor_copy(acc, p_ps[0:1, :])
                nc.vector.tensor_add(acc, acc, p_ps[32:33, :])
                nc.vector.tensor_add(acc, acc, p_ps[64:65, :])
                nc.vector.tensor_add(
                    pooled_sb[:, p["bag"] * L:(p["bag"] + 1) * L],
                    acc, p_ps[96:97, :])

            for it, bag in enumerate(bags):
                xt_sb = []
                for q in range(NQ):
                    if it == 0 and q == 0:
                        xt_sb.append(xt_pre)
                        continue
                    t_ = xt_pool.tile([128, LC, QW], BF16, tag=f"xt{q}")
                    nc.sync.dma_start(
                        out=t_,
                        in_=xT[bag, :, q * QW:(q + 1) * QW]
                        .rearrange("(c p) n -> p c n", p=128),
                    )
                    xt_sb.append(t_)
                # natural-layout x for the PENDING bag's pooled matmuls (which
                # run during this bag's mains) — deferring it keeps this bag's
                # xT quarters at the head of the DMA queue
                if pend or not do_rest:
                    xn_sb = xn_pool.tile([128, NT, L], BF16, tag="xn")
                    nc.sync.dma_start(
                        out=xn_sb,
                        in_=xn[pend["bag"] if pend else bag]
                        .rearrange("(t p) l -> p t l", p=128))
                    if pend:
                        pend["xn"] = xn_sb
                if do_rest and it == len(bags) - 1:
                    # the final bag's xn, loaded now so its pooled tail
                    # (emitted after the loop) doesn't wait on DMA
                    xn_last = xn_pool.tile([128, NT, L], BF16, tag="xn")
                    nc.sync.dma_start(
                        out=xn_last,
                        in_=xn[bag].rearrange("(t p) l -> p t l", p=128))

                a_sb = acts.tile([128, DC, N], BF16, tag="a")
                t_sb = acts.tile([128, DC, N], BF16, tag="t")
                hsum = None
                if do_rest:
                    hsum = hsums.tile([128, N], BF16, tag="hsum")
                # main matmuls: out[d, n] = sum_l W[l, d] * xT[l, n]
                for wi in range(2):
                    if not do_mm:
                        break
                    dst = a_sb if wi == 0 else t_sb
                    scale = 1.0 if wi == 0 else 0.5
                    for dc in range(DC):
                        for q in range(NQ):
                            ps = pmain.tile([128, 2, 512], F32)
                            for lc in range(LC):
                                for i in range(2):
                                    nc.tensor.matmul(
                                        ps[:, i],
                                        lhsT=w_sb[:, wi, lc,
                                                  dc * 128:(dc + 1) * 128],
                                        rhs=xt_sb[q][:, lc,
                                                     i * 512:(i + 1) * 512],
                                        start=(lc == 0), stop=(lc == LC - 1),
                                    )
                            if do_act:
                                nc.scalar.activation(
                                    out=dst[:, dc, q * QW:(q + 1) * QW],
                                    in_=ps[:].rearrange("p a b -> p (a b)"),
                                    func=mybir.ActivationFunctionType.Tanh,
                                    bias=bias_sb[:, wi, dc:dc + 1],
                                    scale=scale,
                                )
                        # previous bag's score emitted at the 25% point of
                        # this bag's mains: spreads the N=1 score matmuls away
                        # from the pooled block and completes exp/e_bf earlier
                        if (do_rest and pend and wi == 0 and dc == 0
                                and not pend.get("scored")):
                            emit_score(pend)
                            pend["scored"] = True
                        # gating as soon as this dc's tanh pair is complete:
                        # h = a*(1+t), hsum += h*wc_dc   (all bf16, in place;
                        # last bag keeps the dc parts separate so emit_score
                        # can start on the dc=0 half sooner)
                        if do_rest and wi == 1:
                            last = it == len(bags) - 1
                            nc.vector.tensor_mul(t_sb[:, dc], a_sb[:, dc],
                                                 t_sb[:, dc])
                            nc.vector.tensor_add(t_sb[:, dc], a_sb[:, dc],
                                                 t_sb[:, dc])
                            if dc == 0:
                                nc.vector.tensor_scalar_mul(
                                    hsum, t_sb[:, 0], wc_sb[:, 0:1])
                            else:
                                nc.vector.tensor_scalar_mul(
                                    t_sb[:, 1], t_sb[:, 1], wc_sb[:, 1:2])
                                nc.vector.tensor_add(hsum, hsum,
                                                     t_sb[:, 1])
                    # interleave the previous bag's tail into this bag's mains
                    if do_rest and pend:
                        if wi == 0 and not pend.get("scored"):
                            emit_score(pend)
                            pend["scored"] = True
                        elif wi == 1:
                            emit_pooled(pend)

                if not do_rest:
                    continue

                pend = {"bag": bag, "hsum": hsum}

            if do_rest and pend:
                pend["xn"] = xn_last
                emit_score(pend)
                emit_pooled(pend)
            nc.sync.dma_start(out=pooled_out, in_=pooled_sb)
    nc.finalize()
    return nc


def _get_nc():
    if "nc" not in _cache:
        _cache["nc"] = _build()
    return _cache["nc"]


def kernel(x_batch, Wa, ba, Wb, bb, Wc, bc):
    x_batch = np.asarray(x_batch)
    Wa, ba = np.asarray(Wa, np.float32), np.asarray(ba, np.float32)
    Wb, bb = np.asarray(Wb, np.float32), np.asarray(bb, np.float32)
    Wc = np.asarray(Wc, np.float32)
    in_dtype = x_batch.dtype
    xf = np.asarray(x_batch, np.float32)

    bf16 = ml_dtypes.bfloat16
    # stationary weights: [p, wi, lc, d] = W_wi[lc*128+p, d]
    wstack = np.stack([Wa, Wb]).reshape(2, LC, 128, D).transpose(2, 0, 1, 3)
    w_host = np.ascontiguousarray(wstack).astype(bf16)
    # bias: [p, wi, dc]; the b-branch uses tanh(z/2) -> bias = bb/2
    bias_host = np.stack([ba, 0.5 * bb]).reshape(2, DC, 128).transpose(2, 0, 1)
    bias_host = np.ascontiguousarray(bias_host, np.float32)
    # wc: [p, dc] = 0.5 * Wc[dc*128+p]  (0.5 from the sigmoid-as-tanh identity)
    wc_host = np.ascontiguousarray(
        (0.5 * Wc[:, 0]).reshape(DC, 128).T, dtype=np.float32)

    xb = xf.astype(bf16)                      # [B, N, L]
    in_maps = []
    for c in range(NCORES):
        sh = xb[c * BAGS:(c + 1) * BAGS]
        in_maps.append({
            "xT": np.ascontiguousarray(sh.transpose(0, 2, 1)),
            "xn": np.ascontiguousarray(sh),
            "w": w_host,
            "bias": bias_host,
            "wc": wc_host,
        })

    nc = _get_nc()
    res = bass_utils.run_bass_kernel_spmd(nc, in_maps,
                                          core_ids=list(range(NCORES)))

    A = np.empty((B, N), np.float32)
    pooled = np.empty((B, L), np.float32)
    for c in range(NCORES):
        r = res.results[c]
        e = r["e_out"]                        # [BAGS, 128, NT]
        z = r["z_out"]                        # [BAGS, 128, 1]
        pr = r["pooled_out"].reshape(BAGS, L)
        for b in range(BAGS):
            bag = c * BAGS + b
            Z = z[b].sum(dtype=np.float64)
            A[bag] = (e[b].T.reshape(N) / Z).astype(np.float32)
            pooled[bag] = (pr[b] / Z).astype(np.float32)

    return (A.astype(in_dtype, copy=False), x_batch,
            pooled.astype(in_dtype, copy=False))


if __name__ == "__main__":
    rng = np.random.default_rng(0)
    inputs = {
        "x_batch": rng.standard_normal((B, N, L), dtype=np.float32),
        "Wa": (rng.uniform(-1, 1, (L, D)) / np.sqrt(L)).astype(np.float32),
        "ba": (rng.uniform(-1, 1, (D,)) / np.sqrt(L)).astype(np.float32),
        "Wb": (rng.uniform(-1, 1, (L, D)) / np.sqrt(L)).astype(np.float32),
        "bb": (rng.uniform(-1, 1, (D,)) / np.sqrt(L)).astype(np.float32),
        "Wc": (rng.uniform(-1, 1, (D, 1)) / np.sqrt(D)).astype(np.float32),
        "bc": (rng.uniform(-1, 1, (1,)) / np.sqrt(D)).astype(np.float32),
    }
    A, xb, pooled = kernel(**inputs)
    print("A", A.shape, A.dtype, "pooled", pooled.shape, pooled.dtype)
